# revision 31
# baseline (speedup 1.0000x reference)
"""Trainium2 Bass kernel for nn_ExpNegL2 (exp(-||a_n - t_n||) retrieval scores).

Full inputs: audio [32, 4096, 512] f32, text [32, 64, 512] f32.
Output: [32, 64, 4096] f32 = exp(-sqrt(2 - 2 * <normalize(text), normalize(audio)>)).

Sharding: data-parallel over batch, 4 batches per core across 8 cores.

v5: 2-batch interleave + software-pipelined emission + PE transposes.
Measured 134.5us/body on HW (interleaved repeat-differencing), rel err 5e-4.

Per step = one 512-row t-block of a batch PAIR (two units, h=0/1):
  SWDGE (gpsimd) DMA cast-loads audio fp32->bf16 in natural [t, d] layout
  (partition p holds rows t0+4p..4p+3, 4KB contiguous bf16 per partition) ->
  row ssq split between ACT (fused square+accum per subtile) and DVE (one
  tensor_mul + grouped reduce over the rest), both units' norms in one tile
  -> one Ln + one Exp on ACT give 1/||a|| -> 4 DVE tensor_scalar normalize
  slices per unit (bf16 4x mode) -> transpose to [d, (n,c,t)] bf16 on the
  PE (16 is_transpose matmuls against an identity into a 2-bank bf16 PSUM
  tile + one PSUM->SBUF copy-cast, alternating ACT/DVE per unit; beats the
  DMA xbar because loads+stores already saturate the shared DMA pool) ->
  PE matmul against the two batches' normalized+transposed text [d, m],
  K=512 in 4 chunks, both batches packed into partition halves of ONE PSUM
  tile via tile_position, even/odd t-blocks sharing a 2-bank tile -> ACT
  post over the (even, odd) pair in full [128, 2, 512] ops: Ln / Exp / Exp
  computes exp(-sqrt(2-2s)), the final Exp's store AP undoing the (n,p) row
  permutation -> one DMA store per t-block pair (4KB-contiguous rows).

Emission is explicitly software-pipelined over a linear step index (skew:
load i | ssq i-2 | inv i-3 | norm+transpose i-4 | matmul i-5 | post i-6 |
store last). Each engine executes its stream IN ORDER behind a 4-deep wait
queue, so an instruction waiting on a cross-engine semaphore head-of-line
blocks everything behind it; the skew gives every cross-engine dependency
a full step (~4us) of slack. Stores are emitted last so they never block
the PE's feeders in the SP queue.

A single pre-placed LoadActFuncSet(natural_log_exp_and_others) covers
Square+Ln+Exp: without it the compiler's table pass thrashes 81 reloads
(1283ns each) alternating natural_log / exp_and_others.

Hardware gotchas baked in:
  - tensor_tensor_reduce crashes the device (NRT unrecoverable) - avoided.
  - GPSIMD cannot access PSUM (walrus birverifier) and DMA cannot read
    PSUM, so the PE-transpose copy-out runs on ACT/DVE only.
  - the is_transpose rhs must be a permutation matrix - the transpose
    cannot be fused into the dots matmul.
  - Sqrt and Exp never share an ACT table on trn2 (and no table has
    ln+sqrt+exp), hence sqrt = Exp(0.5*Ln) keeps ACT on one table.
"""

import os
import sys

sys.path.insert(0, "/opt/trn_rl_repo")

import contextlib

import numpy as np

import concourse.bacc as bacc
import concourse.tile as tile
from concourse import mybir
from concourse import bass_utils


def _env(name, default):
    return int(os.environ.get(name, default))


N_CORES = 8
B, T, M, D = 32, 4096, 64, 512
B_LOC = B // N_CORES          # batches per core
TB = 512                      # audio rows per t-block
NT = T // TB                  # t-blocks per batch
NSUB = TB // 128              # 128-row sub-tiles per t-block
NCH = D // 128                # 128-wide contraction chunks
NPAIR = B_LOC // 2            # batch pairs per core

# subtiles per unit squared on ACT (fused square+accum); rest on DVE
N_SQ_ACT = _env("KN_SQACT", 2)
N_SQ_ACT_H = (_env("KN_SQACT0", N_SQ_ACT), _env("KN_SQACT1", N_SQ_ACT))
# subtiles (of each unit's ACT share) offloaded to Pool as mul+reduce pairs
N_SQ_POOL = _env("KN_SQPOOL", 0)
# units per step (0..2) transposed on the PE (is_transpose matmul + identity)
# instead of the DMA xbar; frees DMA-pool time at the cost of PE time and
# PSUM->SBUF copy passes.
N_PE_TR = _env("KN_PETR", 2)
# engines for the 4 per-unit PSUM->SBUF copy passes of a PE transpose
PE_TR_COPY = os.environ.get("KN_PETRCP", "da")
# batch post/store over (even, odd) t-block pairs: fewer, bigger ACT ops and
# 4KB-row stores, at the cost of PSUM tile parallelism. Wins when ACT-bound
# (PE-transpose mode); loses when PSUM-bound (xbar mode).
POST2 = _env("KN_POST2", 1 if N_PE_TR else 0)

F32 = mybir.dt.float32
BF16 = mybir.dt.bfloat16


def _body(ctx, tc, out, audio, text, repeat=1, ablate=()):
    nc = tc.nc
    Ln = mybir.ActivationFunctionType.Ln
    Exp = mybir.ActivationFunctionType.Exp
    Square = mybir.ActivationFunctionType.Square

    singles = ctx.enter_context(tc.tile_pool(name="singles", bufs=1))
    two = singles.tile([128, 1], F32)
    nc.vector.memset(two, 2.0)

    tx_pool = ctx.enter_context(tc.tile_pool(name="tx", bufs=_env("KB_TX", 3)))
    tnt_pool = ctx.enter_context(
        tc.tile_pool(name="tnt", bufs=min(B_LOC * repeat, 2 * B_LOC)))
    nat_pool = ctx.enter_context(tc.tile_pool(name="nat", bufs=_env("KB_NAT", 12)))
    sq_pool = ctx.enter_context(tc.tile_pool(name="sq", bufs=_env("KB_SQ", 4)))
    small_pool = ctx.enter_context(
        tc.tile_pool(name="small", bufs=_env("KB_SMALL", 8)))
    natn_pool = ctx.enter_context(
        tc.tile_pool(name="natn", bufs=_env("KB_NATN", 6)))
    at_pool = ctx.enter_context(tc.tile_pool(name="at", bufs=_env("KB_AT", 8)))
    post_pool = ctx.enter_context(
        tc.tile_pool(name="post", bufs=_env("KB_POST", 4)))
    psum_pool = ctx.enter_context(
        tc.tile_pool(name="psum", bufs=_env("KB_PSUM", 8 if N_PE_TR == 0 else 2),
                     space="PSUM"))
    if N_PE_TR:
        psum_tr_pool = ctx.enter_context(
            tc.tile_pool(name="psumtr", bufs=_env("KB_PSUMTR", 2), space="PSUM"))
        ident = singles.tile([128, 128], BF16)
        import concourse.masks as masks
        masks.make_identity(nc, ident)
        cp_eng = {"a": nc.scalar, "d": nc.vector, "v": nc.vector,
                  "p": nc.gpsimd}

    def text_stage(b):
        # load, l2-normalize rows, cast bf16, transpose to [d_rel, c, m]
        txf = tx_pool.tile([M, D], F32)
        nc.sync.dma_start(out=txf, in_=text[b])
        t_scr = tx_pool.tile([M, D], F32)
        t_ssq = tx_pool.tile([M, 1], F32)
        nc.scalar.activation(t_scr, txf, Square, accum_out=t_ssq)
        t_ln = tx_pool.tile([M, 1], F32)
        nc.scalar.activation(t_ln, t_ssq, Ln)
        t_inv = tx_pool.tile([M, 1], F32)
        nc.scalar.activation(t_inv, t_ln, Exp, scale=-0.5)  # 1/||t||
        txn = tx_pool.tile([M, D], BF16)
        nc.vector.tensor_scalar_mul(txn, txf, t_inv)
        tnt = tnt_pool.tile([128, NCH, M], BF16)
        nc.sync.dma_start(out=tnt, in_=txn, transpose=True)
        return tnt

    # ---- pipeline state per step (q, tb): dicts keyed by step index
    steps = [(r, q, tb)
             for r in range(repeat) for q in range(NPAIR) for tb in range(NT)]
    NS = len(steps)
    nat_t, ssq_t, inv_t, at_t, dots_t = {}, {}, {}, {}, {}
    tnts = {}

    def st_load(i):
        _, q, tb = steps[i]
        b0 = 2 * q
        tiles = []
        for h in (0, 1):
            src = audio[b0 + h, tb * TB:(tb + 1) * TB, :].rearrange(
                "(p n) d -> p n d", p=128
            )
            nat = nat_pool.tile([128, NSUB, D], BF16)
            if "noload" in ablate:
                nc.vector.memset(nat[:, 0, 0:1], 0.5)
            else:
                nc.gpsimd.dma_start(out=nat, in_=src)
            tiles.append(nat)
        nat_t[i] = tiles

    def st_ssq(i):
        # per-row sum of squares for BOTH units into one [128, 2, NSUB, 1]
        # tile, ACT/DVE split per unit
        ssq = small_pool.tile([128, 2, NSUB, 1], F32)
        if "nonorm" in ablate:
            nc.vector.memset(ssq, 512.0)
        else:
            for h in (0, 1):
                nat = nat_t[i][h]
                k = N_SQ_ACT_H[h]
                for n in range(k):
                    if n < N_SQ_POOL:
                        sqp = sq_pool.tile([128, D], BF16)
                        nc.gpsimd.tensor_mul(sqp, nat[:, n, :], nat[:, n, :])
                        nc.vector.reduce_sum(
                            ssq[:, h, n, :], sqp, axis=mybir.AxisListType.X)
                    else:
                        sq_scr = sq_pool.tile([128, D], BF16)
                        nc.scalar.activation(
                            sq_scr, nat[:, n, :], Square,
                            accum_out=ssq[:, h, n, 0:1]
                        )
                if k < NSUB:
                    sq = sq_pool.tile([128, NSUB - k, D], BF16)
                    nc.vector.tensor_mul(sq, nat[:, k:, :], nat[:, k:, :])
                    nc.vector.reduce_sum(
                        ssq[:, h, k:, :], sq, axis=mybir.AxisListType.X
                    )
        ssq_t[i] = ssq

    def st_inv(i):
        # one Ln + one Exp covering both units' [2, NSUB] norms
        rs = small_pool.tile([128, 2, NSUB, 1], F32)
        inv = small_pool.tile([128, 2, NSUB, 1], F32)
        if "nonorm" in ablate:
            nc.vector.memset(inv, 1.0)
        else:
            nc.scalar.activation(rs, ssq_t[i], Ln)
            nc.scalar.activation(inv, rs, Exp, scale=-0.5)  # 1/||a||
        inv_t[i] = inv

    def st_norm_tr(i):
        # all normalizes first, then both transposes back-to-back, so the
        # PE (or SP ring in xbar mode) gets both halves' at-tiles with no
        # DVE round trip between them.
        natns = []
        for h in (0, 1):
            nat, inv = nat_t[i][h], inv_t[i]
            natn = natn_pool.tile([128, NSUB, D], BF16)
            if "noscale" in ablate:
                nc.vector.memset(natn[:, 0, 0:1], 0.5)
            else:
                for n in range(NSUB):
                    nc.vector.tensor_scalar_mul(
                        natn[:, n, :], nat[:, n, :], inv[:, h, n]
                    )
            natns.append(natn)
        out_t = []
        for h in (0, 1):
            at = at_pool.tile([128, NSUB, NCH, 128], BF16)
            if "notr" in ablate:
                nc.vector.memset(at[:, 0, 0, 0:1], 0.5)
            elif h < N_PE_TR:
                # PE transpose: out = lhsT.T via identity rhs, 16 [128,128]
                # tiles into one 2-bank bf16 PSUM tile (each matmul stays
                # within a bank), then ONE copy-cast to SBUF on the engine
                # from PE_TR_COPY.
                tp = psum_tr_pool.tile([128, NSUB, NCH, 128], BF16)
                for n in range(NSUB):
                    for c in range(NCH):
                        nc.tensor.matmul(
                            tp[:, n, c, :], natns[h][:, n, c * 128:(c + 1) * 128],
                            ident, is_transpose=True,
                        )
                eng = cp_eng[PE_TR_COPY[(2 * i + h) % len(PE_TR_COPY)]]
                if eng is nc.scalar:
                    eng.copy(at, tp)
                else:
                    eng.tensor_copy(at, tp)
            else:
                nc.sync.dma_start(out=at, in_=natns[h], transpose=True)
            out_t.append(at)
        at_t[i] = out_t
        del nat_t[i], ssq_t[i], inv_t[i]

    def st_mm(i):
        r, q, tb = steps[i]
        # consecutive even/odd t-blocks of a pair share one 2-bank PSUM tile
        # so the post stage can run full [128, 2, TB] ops over both
        if not POST2:
            dots_t[i] = (psum_pool.tile([128, 1, TB], F32, name='dots1'), 0)
        elif tb % 2 == 0:
            dots_t[i] = (psum_pool.tile([128, 2, TB], F32, name='dots2'), 0)
        else:
            dots_t[i] = (dots_t[i - 1][0], 1)
        dots, par = dots_t[i]
        if "nomm" in ablate:
            nc.vector.memset(dots[:, par, 0:1], 0.5)
        else:
            for h in (0, 1):
                tnt = tnts[(r, 2 * q + h)]
                for c in range(NCH):
                    nc.tensor.matmul(
                        dots[h * M:(h + 1) * M, par, :],
                        tnt[:, c, :], at_t[i][h][:, :, c, :],
                        start=(c == 0), stop=(c == NCH - 1),
                        tile_position=(0, h * M),
                    )
        del at_t[i]

    stage_t = {}

    def st_post(i):
        # with POST2: runs on odd t-blocks only, covering the (even, odd)
        # pair in full-width [128, J, TB] ops
        r, q, tb = steps[i]
        J = 2 if POST2 else 1
        if POST2:
            if tb % 2 == 0:
                return
            dots = dots_t.pop(i)[0]
            dots_t.pop(i - 1)
        else:
            dots = dots_t.pop(i)[0]
        if "nopost" in ablate:
            stage = post_pool.tile([128, J, TB], F32, name="stage0")
            nc.vector.memset(stage[:, 0, 0:1], 0.5)
            stage_t[i] = stage
        else:
            lnz = post_pool.tile([128, J, TB], F32, name="lnz")
            nc.scalar.activation(lnz, dots, Ln, bias=two, scale=-2.0)
            dist = post_pool.tile([128, J, TB], F32, name="dist")
            nc.scalar.activation(dist, lnz, Exp, scale=0.5)
            stage = post_pool.tile([128, J, TB], F32, name="stage")
            sdst = stage.rearrange("z j (p n) -> z j n p", n=NSUB)
            dsrc = dist.rearrange("z j (n p) -> z j n p", n=NSUB)
            nc.scalar.activation(sdst, dsrc, Exp, scale=-1.0)
            stage_t[i] = stage

    def st_store(i):
        # Emitted LAST in the iteration: a store at the head of the
        # in-order SP queue (waiting on ACT) would block the xbars behind
        # it, starving the PE. Fires on odd t-blocks, writing the pair's
        # 4KB-contiguous-per-row window.
        r, q, tb = steps[i]
        if POST2 and tb % 2 == 0:
            return
        b0 = 2 * q
        tb0 = (tb - 1) if POST2 else tb
        dst = out[b0:b0 + 2, :, tb0 * TB:(tb + 1) * TB].rearrange(
            "b m t -> (b m) t")
        nc.sync.dma_start(out=dst, in_=stage_t.pop(i))

    # text for all batches up front (tiny, persistent tnt tiles)
    for r in range(repeat):
        for b in range(B_LOC):
            tnts[(r, b)] = text_stage(b)

    # ---- skewed emission: load(i) | ssq(i-2) | inv(i-3) | norm+tr(i-4) |
    # mm(i-5) | post(i-6). Within an iteration the OLDEST stage is emitted
    # first so each engine's in-order stream sees instructions whose
    # dependencies have had the most wall-clock time to resolve (the 4-deep
    # per-engine wait queues head-of-line block otherwise).
    for i in range(NS + 7):
        if 0 <= i - 6 < NS:
            st_post(i - 6)
        if 0 <= i - 5 < NS:
            st_mm(i - 5)
        if 0 <= i - 4 < NS:
            st_norm_tr(i - 4)
        if 0 <= i - 3 < NS:
            st_inv(i - 3)
        if 0 <= i - 2 < NS:
            st_ssq(i - 2)
        if i < NS:
            st_load(i)
        if 0 <= i - 7 < NS:
            st_store(i - 7)


_NC_CACHE = {}


def _preload_act_table(nc):
    """Pre-place one LoadActFuncSet for the table holding Ln+Exp+Square.

    Without this, insert_act_table_loads picks the first table containing
    each function (Ln->natural_log, Exp/Square->exp_and_others) and thrashes
    81 reloads x 1283ns per build. With a table that covers all three loaded
    on every path, the fixpoint pass inserts zero further loads.
    """
    from concourse.hw_specs import get_activation_tables

    need = {
        mybir.ActivationFunctionType.Ln,
        mybir.ActivationFunctionType.Exp,
        mybir.ActivationFunctionType.Square,
    }
    try:
        tables = get_activation_tables(nc.m.arch)
        idx = next(i for i, n in enumerate(tables) if need <= tables[n])
    except Exception:
        idx = 6  # natural_log_exp_and_others in the cayman act_info.json
    inst = mybir.InstLoadActFuncSet(
        name=nc.get_next_instruction_name(), ins=[], outs=[])
    inst.engine = mybir.EngineType.Activation
    inst.act_func_set_id = idx
    nc.register_instruction(inst)
    nc.main_func.blocks[0].instructions.insert(0, inst)


def _build(repeat=1):
    if repeat in _NC_CACHE:
        return _NC_CACHE[repeat]
    nc = bacc.Bacc(
        "TRN2", target_bir_lowering=False, debug=False,
        enable_asserts=False, num_devices=N_CORES,
    )
    audio = nc.dram_tensor("audio", [B_LOC, T, D], F32, kind="ExternalInput").ap()
    text = nc.dram_tensor("text", [B_LOC, M, D], F32, kind="ExternalInput").ap()
    out = nc.dram_tensor("out", [B_LOC, M, T], F32, kind="ExternalOutput").ap()
    with tile.TileContext(nc) as tc:
        with contextlib.ExitStack() as ctx:
            _body(ctx, tc, out, audio, text, repeat=repeat)
    _preload_act_table(nc)
    nc.compile()
    _NC_CACHE[repeat] = nc
    return nc


def kernel(audio: np.ndarray, text: np.ndarray) -> np.ndarray:
    nc = _build()
    in_maps = []
    for i in range(N_CORES):
        sl = slice(i * B_LOC, (i + 1) * B_LOC)
        in_maps.append({
            "audio": np.ascontiguousarray(audio[sl], dtype=np.float32),
            "text": np.ascontiguousarray(text[sl], dtype=np.float32),
        })
    res = bass_utils.run_bass_kernel_spmd(nc, in_maps, core_ids=list(range(N_CORES)))
    return np.concatenate([r["out"] for r in res.results], axis=0)



# revision 32
# speedup vs baseline: 2.7260x; 2.7260x over previous
"""Trainium2 Bass kernel for nn_ExpNegL2 (exp(-||a_n - t_n||) retrieval scores).

Full inputs: audio [32, 4096, 512] f32, text [32, 64, 512] f32.
Output: [32, 64, 4096] f32 = exp(-sqrt(2 - 2 * <normalize(text), normalize(audio)>)).

Sharding: data-parallel over batch, 4 batches per core across 8 cores.

v5: 2-batch interleave + software-pipelined emission + PE transposes.
Measured 134.5us/body on HW (interleaved repeat-differencing), rel err 5e-4.

Per step = one 512-row t-block of a batch PAIR (two units, h=0/1):
  SWDGE (gpsimd) DMA cast-loads audio fp32->bf16 in natural [t, d] layout
  (partition p holds rows t0+4p..4p+3, 4KB contiguous bf16 per partition) ->
  row ssq split between ACT (fused square+accum per subtile) and DVE (one
  tensor_mul + grouped reduce over the rest), both units' norms in one tile
  -> one Ln + one Exp on ACT give 1/||a|| -> 4 DVE tensor_scalar normalize
  slices per unit (bf16 4x mode) -> transpose to [d, (n,c,t)] bf16 on the
  PE (16 is_transpose matmuls against an identity into a 2-bank bf16 PSUM
  tile + one PSUM->SBUF copy-cast, alternating ACT/DVE per unit; beats the
  DMA xbar because loads+stores already saturate the shared DMA pool) ->
  PE matmul against the two batches' normalized+transposed text [d, m],
  K=512 in 4 chunks, both batches packed into partition halves of ONE PSUM
  tile via tile_position, even/odd t-blocks sharing a 2-bank tile -> ACT
  post over the (even, odd) pair in full [128, 2, 512] ops: Ln / Exp / Exp
  computes exp(-sqrt(2-2s)), the final Exp's store AP undoing the (n,p) row
  permutation -> one DMA store per t-block pair (4KB-contiguous rows).

Emission is explicitly software-pipelined over a linear step index (skew:
load i | ssq i-2 | inv i-3 | norm+transpose i-4 | matmul i-5 | post i-6 |
store last). Each engine executes its stream IN ORDER behind a 4-deep wait
queue, so an instruction waiting on a cross-engine semaphore head-of-line
blocks everything behind it; the skew gives every cross-engine dependency
a full step (~4us) of slack. Stores are emitted last so they never block
the PE's feeders in the SP queue.

A single pre-placed LoadActFuncSet(natural_log_exp_and_others) covers
Square+Ln+Exp: without it the compiler's table pass thrashes 81 reloads
(1283ns each) alternating natural_log / exp_and_others.

Hardware gotchas baked in:
  - tensor_tensor_reduce crashes the device (NRT unrecoverable) - avoided.
  - GPSIMD cannot access PSUM (walrus birverifier) and DMA cannot read
    PSUM, so the PE-transpose copy-out runs on ACT/DVE only.
  - the is_transpose rhs must be a permutation matrix - the transpose
    cannot be fused into the dots matmul.
  - Sqrt and Exp never share an ACT table on trn2 (and no table has
    ln+sqrt+exp), hence sqrt = Exp(0.5*Ln) keeps ACT on one table.
"""

import os
import sys

sys.path.insert(0, "/opt/trn_rl_repo")

import contextlib

import numpy as np

import concourse.bacc as bacc
import concourse.tile as tile
from concourse import mybir
from concourse import bass_utils


def _env(name, default):
    return int(os.environ.get(name, default))


N_CORES = 8
B, T, M, D = 32, 4096, 64, 512
B_LOC = B // N_CORES          # batches per core
TB = 512                      # audio rows per t-block
NT = T // TB                  # t-blocks per batch
NSUB = TB // 128              # 128-row sub-tiles per t-block
NCH = D // 128                # 128-wide contraction chunks
NPAIR = B_LOC // 2            # batch pairs per core

# subtiles per unit squared on ACT (fused square+accum); rest on DVE
N_SQ_ACT = _env("KN_SQACT", 2)
N_SQ_ACT_H = (_env("KN_SQACT0", 3), _env("KN_SQACT1", N_SQ_ACT))
# subtiles (of each unit's ACT share) offloaded to Pool as mul+reduce pairs
N_SQ_POOL = _env("KN_SQPOOL", 0)
# units per step (0..2) transposed on the PE (is_transpose matmul + identity)
# instead of the DMA xbar; frees DMA-pool time at the cost of PE time and
# PSUM->SBUF copy passes.
N_PE_TR = _env("KN_PETR", 2)
# engines for the 4 per-unit PSUM->SBUF copy passes of a PE transpose
PE_TR_COPY = os.environ.get("KN_PETRCP", "dd")
# batch post/store over (even, odd) t-block pairs: fewer, bigger ACT ops and
# 4KB-row stores, at the cost of PSUM tile parallelism. Wins when ACT-bound
# (PE-transpose mode); loses when PSUM-bound (xbar mode).
POST2 = _env("KN_POST2", 1 if N_PE_TR else 0)

F32 = mybir.dt.float32
BF16 = mybir.dt.bfloat16


def _body(ctx, tc, out, audio, text, repeat=1, ablate=()):
    nc = tc.nc
    Ln = mybir.ActivationFunctionType.Ln
    Exp = mybir.ActivationFunctionType.Exp
    Square = mybir.ActivationFunctionType.Square

    singles = ctx.enter_context(tc.tile_pool(name="singles", bufs=1))
    two = singles.tile([128, 1], F32)
    nc.vector.memset(two, 2.0)

    tx_pool = ctx.enter_context(tc.tile_pool(name="tx", bufs=_env("KB_TX", 3)))
    tnt_pool = ctx.enter_context(
        tc.tile_pool(name="tnt", bufs=min(B_LOC * repeat, 2 * B_LOC)))
    nat_pool = ctx.enter_context(tc.tile_pool(name="nat", bufs=_env("KB_NAT", 12)))
    sq_pool = ctx.enter_context(tc.tile_pool(name="sq", bufs=_env("KB_SQ", 4)))
    small_pool = ctx.enter_context(
        tc.tile_pool(name="small", bufs=_env("KB_SMALL", 8)))
    natn_pool = ctx.enter_context(
        tc.tile_pool(name="natn", bufs=_env("KB_NATN", 6)))
    at_pool = ctx.enter_context(tc.tile_pool(name="at", bufs=_env("KB_AT", 8)))
    post_pool = ctx.enter_context(
        tc.tile_pool(name="post", bufs=_env("KB_POST", 4)))
    psum_pool = ctx.enter_context(
        tc.tile_pool(name="psum", bufs=_env("KB_PSUM", 8 if N_PE_TR == 0 else 2),
                     space="PSUM"))
    if N_PE_TR:
        psum_tr_pool = ctx.enter_context(
            tc.tile_pool(name="psumtr", bufs=_env("KB_PSUMTR", 2), space="PSUM"))
        ident = singles.tile([128, 128], BF16)
        import concourse.masks as masks
        masks.make_identity(nc, ident)
        cp_eng = {"a": nc.scalar, "d": nc.vector, "v": nc.vector,
                  "p": nc.gpsimd}

    def text_stage(b):
        # load, l2-normalize rows, cast bf16, transpose to [d_rel, c, m]
        txf = tx_pool.tile([M, D], F32)
        nc.sync.dma_start(out=txf, in_=text[b])
        t_scr = tx_pool.tile([M, D], F32)
        t_ssq = tx_pool.tile([M, 1], F32)
        nc.scalar.activation(t_scr, txf, Square, accum_out=t_ssq)
        t_ln = tx_pool.tile([M, 1], F32)
        nc.scalar.activation(t_ln, t_ssq, Ln)
        t_inv = tx_pool.tile([M, 1], F32)
        nc.scalar.activation(t_inv, t_ln, Exp, scale=-0.5)  # 1/||t||
        txn = tx_pool.tile([M, D], BF16)
        nc.vector.tensor_scalar_mul(txn, txf, t_inv)
        tnt = tnt_pool.tile([128, NCH, M], BF16)
        nc.sync.dma_start(out=tnt, in_=txn, transpose=True)
        return tnt

    # ---- pipeline state per step (q, tb): dicts keyed by step index
    steps = [(r, q, tb)
             for r in range(repeat) for q in range(NPAIR) for tb in range(NT)]
    NS = len(steps)
    nat_t, ssq_t, inv_t, at_t, dots_t = {}, {}, {}, {}, {}
    tnts = {}

    def st_load(i):
        _, q, tb = steps[i]
        b0 = 2 * q
        tiles = []
        for h in (0, 1):
            src = audio[b0 + h, tb * TB:(tb + 1) * TB, :].rearrange(
                "(p n) d -> p n d", p=128
            )
            nat = nat_pool.tile([128, NSUB, D], BF16)
            if "noload" in ablate:
                nc.vector.memset(nat[:, 0, 0:1], 0.5)
            else:
                nc.gpsimd.dma_start(out=nat, in_=src)
            tiles.append(nat)
        nat_t[i] = tiles

    def st_ssq(i):
        # per-row sum of squares for BOTH units into one [128, 2, NSUB, 1]
        # tile, ACT/DVE split per unit
        ssq = small_pool.tile([128, 2, NSUB, 1], F32)
        if "nonorm" in ablate:
            nc.vector.memset(ssq, 512.0)
        else:
            for h in (0, 1):
                nat = nat_t[i][h]
                k = N_SQ_ACT_H[h]
                for n in range(k):
                    if n < N_SQ_POOL:
                        sqp = sq_pool.tile([128, D], BF16)
                        nc.gpsimd.tensor_mul(sqp, nat[:, n, :], nat[:, n, :])
                        nc.vector.reduce_sum(
                            ssq[:, h, n, :], sqp, axis=mybir.AxisListType.X)
                    else:
                        sq_scr = sq_pool.tile([128, D], BF16)
                        nc.scalar.activation(
                            sq_scr, nat[:, n, :], Square,
                            accum_out=ssq[:, h, n, 0:1]
                        )
                if k < NSUB:
                    sq = sq_pool.tile([128, NSUB - k, D], BF16)
                    nc.vector.tensor_mul(sq, nat[:, k:, :], nat[:, k:, :])
                    nc.vector.reduce_sum(
                        ssq[:, h, k:, :], sq, axis=mybir.AxisListType.X
                    )
        ssq_t[i] = ssq

    def st_inv(i):
        # one Ln + one Exp covering both units' [2, NSUB] norms
        rs = small_pool.tile([128, 2, NSUB, 1], F32)
        inv = small_pool.tile([128, 2, NSUB, 1], F32)
        if "nonorm" in ablate:
            nc.vector.memset(inv, 1.0)
        else:
            nc.scalar.activation(rs, ssq_t[i], Ln)
            nc.scalar.activation(inv, rs, Exp, scale=-0.5)  # 1/||a||
        inv_t[i] = inv

    def st_norm_tr(i):
        # all normalizes first, then both transposes back-to-back, so the
        # PE (or SP ring in xbar mode) gets both halves' at-tiles with no
        # DVE round trip between them.
        natns = []
        for h in (0, 1):
            nat, inv = nat_t[i][h], inv_t[i]
            natn = natn_pool.tile([128, NSUB, D], BF16)
            if "noscale" in ablate:
                nc.vector.memset(natn[:, 0, 0:1], 0.5)
            else:
                for n in range(NSUB):
                    nc.vector.tensor_scalar_mul(
                        natn[:, n, :], nat[:, n, :], inv[:, h, n]
                    )
            natns.append(natn)
        out_t = []
        for h in (0, 1):
            at = at_pool.tile([128, NSUB, NCH, 128], BF16)
            if "notr" in ablate:
                nc.vector.memset(at[:, 0, 0, 0:1], 0.5)
            elif h < N_PE_TR:
                # PE transpose: out = lhsT.T via identity rhs, 16 [128,128]
                # tiles into one 2-bank bf16 PSUM tile (each matmul stays
                # within a bank), then ONE copy-cast to SBUF on the engine
                # from PE_TR_COPY.
                tp = psum_tr_pool.tile([128, NSUB, NCH, 128], BF16)
                for n in range(NSUB):
                    for c in range(NCH):
                        nc.tensor.matmul(
                            tp[:, n, c, :], natns[h][:, n, c * 128:(c + 1) * 128],
                            ident, is_transpose=True,
                        )
                eng = cp_eng[PE_TR_COPY[(2 * i + h) % len(PE_TR_COPY)]]
                if eng is nc.scalar:
                    eng.copy(at, tp)
                else:
                    eng.tensor_copy(at, tp)
            else:
                nc.sync.dma_start(out=at, in_=natns[h], transpose=True)
            out_t.append(at)
        at_t[i] = out_t
        del nat_t[i], ssq_t[i], inv_t[i]

    def st_mm(i):
        r, q, tb = steps[i]
        # consecutive even/odd t-blocks of a pair share one 2-bank PSUM tile
        # so the post stage can run full [128, 2, TB] ops over both
        if not POST2:
            dots_t[i] = (psum_pool.tile([128, 1, TB], F32, name='dots1'), 0)
        elif tb % 2 == 0:
            dots_t[i] = (psum_pool.tile([128, 2, TB], F32, name='dots2'), 0)
        else:
            dots_t[i] = (dots_t[i - 1][0], 1)
        dots, par = dots_t[i]
        if "nomm" in ablate:
            nc.vector.memset(dots[:, par, 0:1], 0.5)
        else:
            for h in (0, 1):
                tnt = tnts[(r, 2 * q + h)]
                for c in range(NCH):
                    nc.tensor.matmul(
                        dots[h * M:(h + 1) * M, par, :],
                        tnt[:, c, :], at_t[i][h][:, :, c, :],
                        start=(c == 0), stop=(c == NCH - 1),
                        tile_position=(0, h * M),
                    )
        del at_t[i]

    stage_t = {}

    def st_post(i):
        # with POST2: runs on odd t-blocks only, covering the (even, odd)
        # pair in full-width [128, J, TB] ops
        r, q, tb = steps[i]
        J = 2 if POST2 else 1
        if POST2:
            if tb % 2 == 0:
                return
            dots = dots_t.pop(i)[0]
            dots_t.pop(i - 1)
        else:
            dots = dots_t.pop(i)[0]
        if "nopost" in ablate:
            stage = post_pool.tile([128, J, TB], F32, name="stage0")
            nc.vector.memset(stage[:, 0, 0:1], 0.5)
            stage_t[i] = stage
        else:
            lnz = post_pool.tile([128, J, TB], F32, name="lnz")
            nc.scalar.activation(lnz, dots, Ln, bias=two, scale=-2.0)
            dist = post_pool.tile([128, J, TB], F32, name="dist")
            nc.scalar.activation(dist, lnz, Exp, scale=0.5)
            stage = post_pool.tile([128, J, TB], F32, name="stage")
            sdst = stage.rearrange("z j (p n) -> z j n p", n=NSUB)
            dsrc = dist.rearrange("z j (n p) -> z j n p", n=NSUB)
            nc.scalar.activation(sdst, dsrc, Exp, scale=-1.0)
            stage_t[i] = stage

    def st_store(i):
        # Emitted LAST in the iteration: a store at the head of the
        # in-order SP queue (waiting on ACT) would block the xbars behind
        # it, starving the PE. Fires on odd t-blocks, writing the pair's
        # 4KB-contiguous-per-row window.
        r, q, tb = steps[i]
        if POST2 and tb % 2 == 0:
            return
        b0 = 2 * q
        tb0 = (tb - 1) if POST2 else tb
        dst = out[b0:b0 + 2, :, tb0 * TB:(tb + 1) * TB].rearrange(
            "b m t -> (b m) t")
        nc.sync.dma_start(out=dst, in_=stage_t.pop(i))

    # text for all batches up front (tiny, persistent tnt tiles)
    for r in range(repeat):
        for b in range(B_LOC):
            tnts[(r, b)] = text_stage(b)

    # ---- skewed emission: load(i) | ssq(i-2) | inv(i-3) | norm+tr(i-4) |
    # mm(i-5) | post(i-6). Within an iteration the OLDEST stage is emitted
    # first so each engine's in-order stream sees instructions whose
    # dependencies have had the most wall-clock time to resolve (the 4-deep
    # per-engine wait queues head-of-line block otherwise).
    for i in range(NS + 7):
        if 0 <= i - 6 < NS:
            st_post(i - 6)
        if 0 <= i - 5 < NS:
            st_mm(i - 5)
        if 0 <= i - 4 < NS:
            st_norm_tr(i - 4)
        if 0 <= i - 3 < NS:
            st_inv(i - 3)
        if 0 <= i - 2 < NS:
            st_ssq(i - 2)
        if i < NS:
            st_load(i)
        if 0 <= i - 7 < NS:
            st_store(i - 7)


_NC_CACHE = {}


def _preload_act_table(nc):
    """Pre-place one LoadActFuncSet for the table holding Ln+Exp+Square.

    Without this, insert_act_table_loads picks the first table containing
    each function (Ln->natural_log, Exp/Square->exp_and_others) and thrashes
    81 reloads x 1283ns per build. With a table that covers all three loaded
    on every path, the fixpoint pass inserts zero further loads.
    """
    from concourse.hw_specs import get_activation_tables

    need = {
        mybir.ActivationFunctionType.Ln,
        mybir.ActivationFunctionType.Exp,
        mybir.ActivationFunctionType.Square,
    }
    try:
        tables = get_activation_tables(nc.m.arch)
        idx = next(i for i, n in enumerate(tables) if need <= tables[n])
    except Exception:
        idx = 6  # natural_log_exp_and_others in the cayman act_info.json
    inst = mybir.InstLoadActFuncSet(
        name=nc.get_next_instruction_name(), ins=[], outs=[])
    inst.engine = mybir.EngineType.Activation
    inst.act_func_set_id = idx
    nc.register_instruction(inst)
    nc.main_func.blocks[0].instructions.insert(0, inst)


def _build(repeat=1):
    if repeat in _NC_CACHE:
        return _NC_CACHE[repeat]
    nc = bacc.Bacc(
        "TRN2", target_bir_lowering=False, debug=False,
        enable_asserts=False, num_devices=N_CORES,
    )
    audio = nc.dram_tensor("audio", [B_LOC, T, D], F32, kind="ExternalInput").ap()
    text = nc.dram_tensor("text", [B_LOC, M, D], F32, kind="ExternalInput").ap()
    out = nc.dram_tensor("out", [B_LOC, M, T], F32, kind="ExternalOutput").ap()
    with tile.TileContext(nc) as tc:
        with contextlib.ExitStack() as ctx:
            _body(ctx, tc, out, audio, text, repeat=repeat)
    _preload_act_table(nc)
    nc.compile()
    _NC_CACHE[repeat] = nc
    return nc


def kernel(audio: np.ndarray, text: np.ndarray) -> np.ndarray:
    nc = _build()
    in_maps = []
    for i in range(N_CORES):
        sl = slice(i * B_LOC, (i + 1) * B_LOC)
        in_maps.append({
            "audio": np.ascontiguousarray(audio[sl], dtype=np.float32),
            "text": np.ascontiguousarray(text[sl], dtype=np.float32),
        })
    res = bass_utils.run_bass_kernel_spmd(nc, in_maps, core_ids=list(range(N_CORES)))
    return np.concatenate([r["out"] for r in res.results], axis=0)

